# revision 41
# baseline (speedup 1.0000x reference)
"""Trainium2 Bass kernel for PoissonGaussianReadout (v2).

Computation (per reference):
  out[b, n] = elu( sum_c bilinear_sample(x[b, c], mu[n]) * W[n, c] + bias[n] ) + 1

Sharding: data-parallel over batch B=32 across 8 cores (4 images per core).
Every core processes all N=8192 neurons for its 4 images.

Device strategy per core ("rect-B" v2):
  - Neurons are clustered into 64 tiles of 128 via k-d median splits
    (first two splits on y => 4 y-bands), giving ONE small rect per tile
    (fd = nr*xl <= 128).
  - x is host-trimmed to the used pixel bbox (rows/cols 2..61) and packed
    per y-band (halo rows duplicated) so each band is one contiguous DMA.
  - W is quantized to fp8 e3m4 with a per-neuron scale folded into S'.
  - TensorE: per (tile, ch): one matmul Y[n, 4b, fd] += W_chunk^T @ x_rect,
    PSUM pair-units [P, 2, 4, 128] = 2 banks, 4 bufs in flight.
  - Per pair, one of two lanes:
      VE lane: ScalarE drains PSUM->SBUF bf16, DVE tensor_tensor (*S', 2x)
               + tensor_reduce -> z.
      GpSimd lane: gpsimd tensor_tensor reads PSUM f32 directly (*S')
               + gpsimd tensor_reduce -> z (no scalar drain).
  - Epilogue (two halves, interleaved): z += bias; out = elu(z)+1.
"""

import numpy as np
import ml_dtypes

B, C, H, Wd, N = 32, 256, 64, 64, 8192
NCORES = 8
BL = B // NCORES          # 4 images per core
P = 128                   # partitions / neurons per tile
NT = N // P               # 64 neuron tiles
NPAIR = NT // 2

W_FP8 = True              # W in fp8 e3m4 (scale folded into S')
X_FP8 = False             # x in fp8 e3m4 (halves the dominant DMA stream)
NQUAD = NT // 4           # 16 quads of 4 tiles
GPS_TT_QUADS = frozenset(
    q for q in range(NQUAD) if q % 3 != 2 and q < 14
)  # TT on GpSimd; last quads stay on VE to shorten the tail
ACT_QUADS = frozenset()   # reduce via scalar activation accumulate (slow; off)
FDQ_CAP = 128

_PROGRAM = None


def _build_program(meta):
    import concourse.bass as bass
    import concourse.mybir as mybir
    import concourse.tile as tile

    bf16 = mybir.dt.bfloat16
    fp8 = mybir.dt.float8e3
    f32 = mybir.dt.float32
    wdt = fp8 if W_FP8 else bf16
    xdt = fp8 if X_FP8 else bf16

    rects = meta["rects"]        # per tile: (band, rmin_local, nr, xmin, xl)
    fdt = meta["fdt"]            # per tile fd
    fdq = meta["fdq"]            # per pair padded fd
    soff = meta["soff"]          # per pair S' offset (elems per partition)
    ssz = meta["ssz"]
    bandrows = meta["bandrows"]  # rows per band block (with halo)
    bandoff = meta["bandoff"]    # row offset of each band block in xt
    XROWS = sum(bandrows)
    XC = meta["xc"]              # trimmed col count

    nc = bass.Bass("TRN2")

    xt = nc.dram_tensor("xt", [P, 2 * BL * XROWS * XC], xdt, kind="ExternalInput")
    ws = nc.dram_tensor("ws", [P, NT * 2 * P], wdt, kind="ExternalInput")
    ss = nc.dram_tensor("ss", [P, ssz], bf16, kind="ExternalInput")
    biasr = nc.dram_tensor("biasr", [P, NT], f32, kind="ExternalInput")
    out = nc.dram_tensor("out", [P, NT * BL], f32, kind="ExternalOutput")

    with tile.TileContext(nc) as tc:
        with (
            tc.tile_pool(name="const", bufs=1) as cpool,
            tc.tile_pool(name="fpool", bufs=4) as fpool,
            tc.tile_pool(name="upool", bufs=4) as upool,
            tc.tile_pool(name="psum", bufs=2, space="PSUM") as ppool,
        ):
            # per-(band, ch) x tiles: fully contiguous DMA on both sides
            NB = len(bandrows)
            x_bc = [
                [
                    cpool.tile(
                        [P, BL, bandrows[bd], XC], xdt,
                        name=f"x_b{bd}c{ch}",
                    )
                    for ch in range(2)
                ]
                for bd in range(NB)
            ]
            s_sb = cpool.tile([P, ssz], bf16)
            w_sb = cpool.tile([P, NT, 2, P], wdt)
            bias_sb = cpool.tile([P, NT], f32)
            z_sb = cpool.tile([P, NT, BL], f32)

            # DMA per band: W and S' first (small; clears tile deps), then
            # the two x channel blocks.  Band-major tile ordering makes each
            # transfer a single contiguous line per partition.
            bandt0 = meta["bandt0"]
            xoff = 0
            for bd in range(NB):
                t0, t1 = bandt0[bd], bandt0[bd + 1]
                blk = BL * bandrows[bd] * XC
                nc.sync.dma_start(
                    w_sb[:, t0:t1].rearrange("p t c n -> p (t c n)"),
                    ws[:, t0 * 2 * P : t1 * 2 * P],
                )
                s0, s1 = soff[t0 // 4], soff[t1 // 4]
                nc.sync.dma_start(s_sb[:, s0:s1], ss[:, s0:s1])
                for ch in range(2):
                    nc.sync.dma_start(
                        x_bc[bd][ch][:].rearrange("p b r x -> p (b r x)"),
                        xt[:, xoff : xoff + blk],
                    )
                    xoff += blk
            nc.sync.dma_start(bias_sb[:], biasr[:])

            # pre-zero the PSUM pool buffers: drains/TTs read pad columns no
            # matmul ever writes; virgin PSUM could be NaN.
            for _ in range(4):
                pz = ppool.tile([P, 2, BL, FDQ_CAP], f32, tag="ps")
                nc.scalar.mul(
                    pz[:].rearrange("p a b c -> p (a b c)"),
                    pz[:].rearrange("p a b c -> p (a b c)"),
                    0.0,
                )

            zflat = z_sb[:].rearrange("p t b -> p (t b)")
            trash = cpool.tile([P, FDQ_CAP], bf16)
            pending_act = []  # deferred scalar act-reduces: (u, Q, fq)
            pending_red = []  # deferred VE reduces for GpSimd quads: (u, Q)

            def flush_act():
                while pending_act:
                    pu, pq, pfq = pending_act.pop(0)
                    for tq in range(4):
                        t = 4 * pq + tq
                        for bb in range(BL):
                            nc.scalar.activation(
                                out=trash[:, 0:pfq],
                                in_=pu[:, tq, bb],
                                func=mybir.ActivationFunctionType.Copy,
                                accum_out=zflat[
                                    :, t * BL + bb : t * BL + bb + 1
                                ],
                            )

            for Q in range(NQUAD):
                fq = fdq[Q]
                f_bf = fpool.tile([P, 4, BL, fq], bf16, tag="f")
                u = upool.tile([P, 4, BL, fq], bf16, tag="u")
                eng = nc.gpsimd if Q in GPS_TT_QUADS else nc.vector
                s_q = s_sb[:, soff[Q] : soff[Q + 1]].rearrange(
                    "p (t d) -> p t d", t=4
                )
                for half in range(2):
                    # pair-granular PSUM units (2 banks, 4 in flight) keep
                    # the PE runway deep; two drains fill one quad buffer
                    ps = ppool.tile([P, 2, BL, FDQ_CAP], f32, tag="ps")
                    for tp in range(2):
                        tq = 2 * half + tp
                        t = 4 * Q + tq
                        bd, rmin, nr, xmin, xl = rects[t]
                        for ch in range(2):
                            nc.tensor.matmul(
                                ps[:, tp, :, 0 : nr * xl],
                                w_sb[:, t, ch, :],
                                x_bc[bd][ch][
                                    :, :,
                                    rmin : rmin + nr,
                                    xmin : xmin + xl,
                                ],
                                start=(ch == 0),
                                stop=(ch == 1),
                                skip_group_check=True,
                            )
                    hs = slice(2 * half, 2 * half + 2)
                    nc.scalar.copy(f_bf[:, hs], ps[:, :, :, 0:fq])
                    # half-granular TT starts right after its drain
                    eng.tensor_tensor(
                        out=u[:, hs],
                        in0=f_bf[:, hs],
                        in1=s_q[:, hs].unsqueeze(2).broadcast_to(
                            [P, 2, BL, fq]
                        ),
                        op=mybir.AluOpType.mult,
                    )
                    if half == 0:
                        flush_act()
                if Q in ACT_QUADS:
                    pending_act.append((u, Q, fq))
                elif Q in GPS_TT_QUADS:
                    # defer the VE reduce one quad so the in-order VE queue
                    # isn't head-blocked waiting on the slower GpSimd TT
                    pending_red.append((u, Q))
                else:
                    nc.vector.tensor_reduce(
                        out=z_sb[:, 4 * Q : 4 * Q + 4],
                        in_=u[:],
                        axis=mybir.AxisListType.X,
                        op=mybir.AluOpType.add,
                    )
                while len(pending_red) > 1:
                    pu, pq = pending_red.pop(0)
                    nc.vector.tensor_reduce(
                        out=z_sb[:, 4 * pq : 4 * pq + 4],
                        in_=pu[:],
                        axis=mybir.AxisListType.X,
                        op=mybir.AluOpType.add,
                    )

                if Q == NQUAD - 1 or Q == NQUAD // 2 - 1:
                    flush_act()
                    while pending_red:
                        pu, pq = pending_red.pop(0)
                        nc.vector.tensor_reduce(
                            out=z_sb[:, 4 * pq : 4 * pq + 4],
                            in_=pu[:],
                            axis=mybir.AxisListType.X,
                            op=mybir.AluOpType.add,
                        )
                    # epilogue on the completed half: z += bias; elu(z)+1
                    QT = NT // 2
                    h0 = 0 if Q == NQUAD // 2 - 1 else NT // 2
                    ht = slice(h0, h0 + QT)
                    hz = slice(h0 * BL, (h0 + QT) * BL)
                    zf = cpool.tile([P, QT * BL], f32, tag=f"zf{h0}")
                    ze = cpool.tile([P, QT * BL], f32, tag=f"ze{h0}")
                    nc.vector.tensor_tensor(
                        out=zf[:].rearrange("p (t b) -> p t b", b=BL),
                        in0=z_sb[:, ht],
                        in1=bias_sb[:, ht].unsqueeze(-1).broadcast_to(
                            [P, QT, BL]
                        ),
                        op=mybir.AluOpType.add,
                    )
                    # ze = exp(min(zf,0)) = Exp(-Relu(-zf)); zf = Relu(zf)
                    nc.scalar.activation(
                        ze[:], zf[:],
                        mybir.ActivationFunctionType.Relu, scale=-1.0,
                    )
                    nc.scalar.activation(
                        ze[:], ze[:],
                        mybir.ActivationFunctionType.Exp, scale=-1.0,
                    )
                    nc.scalar.activation(
                        zf[:], zf[:], mybir.ActivationFunctionType.Relu
                    )
                    nc.vector.tensor_add(zf[:], zf[:], ze[:])
                    nc.sync.dma_start(out[:, hz], zf[:])

    from concourse.library_overlay import lower_extended_insts

    lower_extended_insts(nc)
    _split_multi_waits(nc)
    nc.finalize()
    return nc


def _split_multi_waits(nc):
    """The walrus build in this environment only supports ONE sync-wait slot
    per instruction.  Hoist extra waits onto NoOps inserted just before the
    offending instruction (same engine, so sequencer order enforces them)."""
    import concourse.mybir as mybir
    import bass_rust

    for fn in nc.m.functions:
        for blk in fn.blocks:
            new_insts = []
            for ins in blk.instructions:
                si = getattr(ins, "sync_info", None)
                waits = list(si.on_wait) if si is not None else []
                if len(waits) > 1:
                    for j, w in enumerate(waits[:-1]):
                        nop = mybir.InstNoOp(name=f"{ins.name}-w{j}")
                        nop.engine = ins.engine
                        nop.sync_info = bass_rust.SyncInfo(
                            on_wait=[w], on_update=[]
                        )
                        new_insts.append(nop)
                    ins.sync_info = bass_rust.SyncInfo(
                        on_wait=[waits[-1]], on_update=list(si.on_update)
                    )
                new_insts.append(ins)
            blk.instructions[:] = new_insts


def _host_prep(x, mu, W, b):
    bf16 = ml_dtypes.bfloat16

    # --- per-neuron bilinear indices / weights ---
    gx = np.clip(mu[:, 0].astype(np.float64), -1.0, 1.0)
    gy = np.clip(mu[:, 1].astype(np.float64), -1.0, 1.0)
    ix = (gx + 1.0) * (Wd * 0.5) - 0.5
    iy = (gy + 1.0) * (H * 0.5) - 0.5
    x0 = np.floor(ix)
    y0 = np.floor(iy)
    wx1 = (ix - x0).astype(np.float32)
    wy1 = (iy - y0).astype(np.float32)
    wx0 = 1.0 - wx1
    wy0 = 1.0 - wy1
    x0i = np.clip(x0.astype(np.int32), 0, Wd - 2)
    y0i = np.clip(y0.astype(np.int32), 0, H - 2)

    # trimmed pixel bbox (cols/rows actually read, incl +1 halo)
    RMIN, RMAX = int(y0i.min()), int(y0i.max()) + 1
    CMIN, CMAX = int(x0i.min()), int(x0i.max()) + 1
    XC = CMAX - CMIN + 1

    # --- k-d clustering: first two splits on y => 4 y-bands of 16 tiles ---
    pts = np.stack([x0i, y0i], 1)

    def bbox_fd(idx):
        xs, ys = pts[idx, 0], pts[idx, 1]
        nr = ys.max() - ys.min() + 2
        xl = xs.max() - xs.min() + 2
        xl += xl & 1
        return nr * xl

    def kd(idx, axes, leaves):
        if len(idx) == P:
            leaves.append(idx)
            return
        h = len(idx) // 2
        if axes:
            ax, rest = axes[0], axes[1:]
        else:
            # pick the split axis minimizing the max child bbox area
            best = None
            for cand in (0, 1):
                o = np.argsort(pts[idx, cand], kind="stable")
                cost = max(bbox_fd(idx[o[:h]]), bbox_fd(idx[o[h:]]))
                if best is None or cost < best[0]:
                    best = (cost, cand)
            ax, rest = best[1], ()
        order = np.argsort(pts[idx, ax], kind="stable")
        kd(idx[order[:h]], rest, leaves)
        kd(idx[order[h:]], rest, leaves)

    leaves = []
    kd(np.arange(N), (1, 1, 1), leaves)  # first three splits on y
    # Re-split the first octant with one more forced y split so the first
    # two bands are 4 tiles each — a smaller first x DMA starts compute
    # earlier.
    oct0 = np.concatenate(leaves[:8])
    sub = []
    kd(oct0, (1,), sub)
    leaves = sub + leaves[8:]
    band_sizes = [4, 4] + [8] * 7
    # Within each band, sort leaves by bbox fd so quads group tiles of
    # similar size (cuts the quad max-pad on S'/DVE work).
    leaves2 = []
    lo = 0
    for bsz in band_sizes:
        grp = sorted(leaves[lo : lo + bsz], key=bbox_fd)
        leaves2.extend(grp)
        lo += bsz
    leaves = leaves2
    order = np.concatenate(leaves)
    y0s, x0s = y0i[order], x0i[order]
    w4 = np.stack(
        [wx0 * wy0, wx1 * wy0, wx0 * wy1, wx1 * wy1], axis=-1
    ).astype(np.float32)[order]

    # --- band row spans (in trimmed coords) incl halo ---
    NB = len(band_sizes)
    bandt0 = np.cumsum([0] + band_sizes).tolist()  # band tile offsets
    bandrows, bandoff, bandr0 = [], [], []
    off = 0
    for bd in range(NB):
        sl = slice(bandt0[bd] * P, bandt0[bd + 1] * P)
        rlo = int(y0s[sl].min()) - RMIN
        rhi = int(y0s[sl].max()) + 1 - RMIN
        bandr0.append(rlo)
        bandoff.append(off)
        bandrows.append(rhi - rlo + 1)
        off += rhi - rlo + 1

    # --- per-tile single rect (local to its band block) ---
    rects, fdt = [], []
    import bisect
    for t in range(NT):
        bd = bisect.bisect_right(bandt0, t) - 1
        sl = slice(t * P, (t + 1) * P)
        yy = y0s[sl] - RMIN - bandr0[bd]   # band-local row of y0
        xx = x0s[sl] - CMIN
        rmin = int(yy.min())
        nr = int(yy.max()) - rmin + 2   # corners reach y0+1
        xmin = int(xx.min())
        xl = int(xx.max()) - xmin + 2   # corners reach x0+1
        # matmul moving rows must be a whole number of 4-byte words
        gran = 4 if X_FP8 else 2
        xl = (xl + gran - 1) & ~(gran - 1)
        if xl > XC - xmin:
            xmin = XC - xl  # shift window left; interior guarantees room
        fd = nr * xl
        assert fd <= FDQ_CAP, (t, nr, xl, fd)
        rects.append((bd, rmin, nr, xmin, xl))
        fdt.append(fd)

    # quad padding for rectangular DVE ops
    fdq = [max(fdt[4 * q : 4 * q + 4]) for q in range(NT // 4)]
    fdq = [f + (f & 1) for f in fdq]
    soff = np.cumsum([0] + [4 * f for f in fdq]).tolist()
    ssz = soff[-1]

    # --- W quantization + per-neuron scale folded into S' ---
    Wp = W[order]  # [N, C] f32
    if W_FP8:
        e3m4 = ml_dtypes.float8_e3m4
        s = np.abs(Wp).max(axis=1) / 15.0  # per-neuron scale
        s = np.maximum(s, 1e-30)
        Wq = (Wp / s[:, None]).astype(e3m4)
        wdt = e3m4
    else:
        s = np.ones(N, dtype=np.float32)
        Wq = Wp.astype(bf16)
        wdt = bf16

    # --- S' (bilinear weights * s[n] over rect cols), pair-padded layout ---
    ss_np = np.zeros((P, ssz), dtype=np.float32)
    for t in range(NT):
        q, tp = t // 4, t % 4
        base = soff[q] + tp * fdq[q]
        bd, rmin, nr, xmin, xl = rects[t]
        sl = slice(t * P, (t + 1) * P)
        yy = y0s[sl] - RMIN - bandr0[bd] - rmin
        xx = x0s[sl] - CMIN - xmin
        sn = s[t * P : (t + 1) * P]
        for j in range(P):
            for (dr, dx, k) in ((0, 0, 0), (0, 1, 1), (1, 0, 2), (1, 1, 3)):
                r, xc = yy[j] + dr, xx[j] + dx
                assert 0 <= r < nr and 0 <= xc < xl, (t, j, r, xc)
                ss_np[j, base + r * xl + xc] += w4[t * P + j, k] * sn[j]
    ss_np = ss_np.astype(bf16)

    # --- W stationary: [c_part, t, ch, n] ---
    ws_np = np.ascontiguousarray(
        Wq.reshape(NT, P, 2, P)        # [t, n, ch, c_part]
        .transpose(3, 0, 2, 1)         # [c_part, t, ch, n]
        .reshape(P, NT * 2 * P)
    )
    biasr_np = np.ascontiguousarray(
        b[order].astype(np.float32).reshape(NT, P).T
    )

    # --- per-core x: band-major blocks [(band, ch), c_part, b, row, col] ---
    xdt = ml_dtypes.float8_e3m4 if X_FP8 else bf16
    xb = x.astype(xdt)[:, :, RMIN : RMAX + 1, CMIN : CMAX + 1]
    xts = []
    for cix in range(NCORES):
        xc_ = xb[cix * BL : (cix + 1) * BL]             # [BL, C, R, XC]
        xc5 = xc_.reshape(BL, 2, P, xc_.shape[2], XC)   # [b, ch, cp, r, x]
        blocks = []
        for bd in range(NB):
            for ch in range(2):
                blk = xc5[:, ch, :, bandr0[bd] : bandr0[bd] + bandrows[bd]]
                # [b, cp, rows, XC] -> [cp, b, rows, XC] flat per partition
                blocks.append(
                    np.ascontiguousarray(blk.transpose(1, 0, 2, 3)).reshape(
                        P, -1
                    )
                )
        xts.append(np.ascontiguousarray(np.concatenate(blocks, axis=1)))

    meta = {
        "rects": rects, "fdt": fdt, "fdq": fdq, "soff": soff, "ssz": ssz,
        "bandrows": bandrows, "bandoff": bandoff, "xc": XC,
        "bandt0": bandt0,
    }
    shared = {"ss": ss_np, "ws": ws_np, "biasr": biasr_np}
    in_maps = [{"xt": xts[cix], **shared} for cix in range(NCORES)]
    return in_maps, meta, order


def _run(prep, trace=False, **kwargs):
    global _PROGRAM
    from concourse import bass_utils

    in_maps, meta, order = prep
    if _PROGRAM is None:
        _PROGRAM = _build_program(meta)
    rr = bass_utils.run_bass_kernel_spmd(
        _PROGRAM, in_maps, core_ids=list(range(NCORES)), trace=trace, **kwargs
    )
    inv = np.empty(N, dtype=np.int64)
    inv[order] = np.arange(N)
    outs = []
    for cix in range(NCORES):
        o = np.asarray(rr.results[cix]["out"], dtype=np.float32)  # [P, NT*BL]
        o = o.reshape(P, NT, BL).transpose(2, 1, 0).reshape(BL, N)
        outs.append(o[:, inv])
    return np.concatenate(outs, axis=0), rr


def kernel(x, mu, W, b):
    prep = _host_prep(x, mu, W, b)
    out, _ = _run(prep)
    return out


# revision 42
# speedup vs baseline: 1.0615x; 1.0615x over previous
"""Trainium2 Bass kernel for PoissonGaussianReadout (v2).

Computation (per reference):
  out[b, n] = elu( sum_c bilinear_sample(x[b, c], mu[n]) * W[n, c] + bias[n] ) + 1

Sharding: data-parallel over batch B=32 across 8 cores (4 images per core).
Every core processes all N=8192 neurons for its 4 images.

Device strategy per core ("rect-B" v2):
  - Neurons are clustered into 64 tiles of 128 via k-d median splits
    (first two splits on y => 4 y-bands), giving ONE small rect per tile
    (fd = nr*xl <= 128).
  - x is host-trimmed to the used pixel bbox (rows/cols 2..61) and packed
    per y-band (halo rows duplicated) so each band is one contiguous DMA.
  - W is quantized to fp8 e3m4 with a per-neuron scale folded into S'.
  - TensorE: per (tile, ch): one matmul Y[n, 4b, fd] += W_chunk^T @ x_rect,
    PSUM pair-units [P, 2, 4, 128] = 2 banks, 4 bufs in flight.
  - Per pair, one of two lanes:
      VE lane: ScalarE drains PSUM->SBUF bf16, DVE tensor_tensor (*S', 2x)
               + tensor_reduce -> z.
      GpSimd lane: gpsimd tensor_tensor reads PSUM f32 directly (*S')
               + gpsimd tensor_reduce -> z (no scalar drain).
  - Epilogue (two halves, interleaved): z += bias; out = elu(z)+1.
"""

import numpy as np
import ml_dtypes

B, C, H, Wd, N = 32, 256, 64, 64, 8192
NCORES = 8
BL = B // NCORES          # 4 images per core
P = 128                   # partitions / neurons per tile
NT = N // P               # 64 neuron tiles
NPAIR = NT // 2

W_FP8 = True              # W in fp8 e3m4 (scale folded into S')
X_FP8 = False             # x in fp8 e3m4 (halves the dominant DMA stream)
NQUAD = NT // 4           # 16 quads of 4 tiles
GPS_TT_QUADS = frozenset()  # TT on GpSimd; empty = all-VE (throttle test)
ACT_QUADS = frozenset()   # reduce via scalar activation accumulate (slow; off)
FDQ_CAP = 128

_PROGRAM = None


def _build_program(meta):
    import concourse.bass as bass
    import concourse.mybir as mybir
    import concourse.tile as tile

    bf16 = mybir.dt.bfloat16
    fp8 = mybir.dt.float8e3
    f32 = mybir.dt.float32
    wdt = fp8 if W_FP8 else bf16
    xdt = fp8 if X_FP8 else bf16

    rects = meta["rects"]        # per tile: (band, rmin_local, nr, xmin, xl)
    fdt = meta["fdt"]            # per tile fd
    fdq = meta["fdq"]            # per pair padded fd
    soff = meta["soff"]          # per pair S' offset (elems per partition)
    ssz = meta["ssz"]
    bandrows = meta["bandrows"]  # rows per band block (with halo)
    bandoff = meta["bandoff"]    # row offset of each band block in xt
    XROWS = sum(bandrows)
    XC = meta["xc"]              # trimmed col count

    nc = bass.Bass("TRN2")

    xt = nc.dram_tensor("xt", [P, 2 * BL * XROWS * XC], xdt, kind="ExternalInput")
    ws = nc.dram_tensor("ws", [P, NT * 2 * P], wdt, kind="ExternalInput")
    ss = nc.dram_tensor("ss", [P, ssz], bf16, kind="ExternalInput")
    biasr = nc.dram_tensor("biasr", [P, NT], f32, kind="ExternalInput")
    out = nc.dram_tensor("out", [P, NT * BL], f32, kind="ExternalOutput")

    with tile.TileContext(nc) as tc:
        with (
            tc.tile_pool(name="const", bufs=1) as cpool,
            tc.tile_pool(name="fpool", bufs=4) as fpool,
            tc.tile_pool(name="upool", bufs=4) as upool,
            tc.tile_pool(name="psum", bufs=2, space="PSUM") as ppool,
        ):
            # per-(band, ch) x tiles: fully contiguous DMA on both sides
            NB = len(bandrows)
            x_bc = [
                [
                    cpool.tile(
                        [P, BL, bandrows[bd], XC], xdt,
                        name=f"x_b{bd}c{ch}",
                    )
                    for ch in range(2)
                ]
                for bd in range(NB)
            ]
            s_sb = cpool.tile([P, ssz], bf16)
            w_sb = cpool.tile([P, NT, 2, P], wdt)
            bias_sb = cpool.tile([P, NT], f32)
            z_sb = cpool.tile([P, NT, BL], f32)

            # DMA per band: W and S' first (small; clears tile deps), then
            # the two x channel blocks.  Band-major tile ordering makes each
            # transfer a single contiguous line per partition.
            bandt0 = meta["bandt0"]
            xoff = 0
            for bd in range(NB):
                t0, t1 = bandt0[bd], bandt0[bd + 1]
                blk = BL * bandrows[bd] * XC
                nc.sync.dma_start(
                    w_sb[:, t0:t1].rearrange("p t c n -> p (t c n)"),
                    ws[:, t0 * 2 * P : t1 * 2 * P],
                )
                s0, s1 = soff[t0 // 4], soff[t1 // 4]
                nc.sync.dma_start(s_sb[:, s0:s1], ss[:, s0:s1])
                for ch in range(2):
                    nc.sync.dma_start(
                        x_bc[bd][ch][:].rearrange("p b r x -> p (b r x)"),
                        xt[:, xoff : xoff + blk],
                    )
                    xoff += blk
            nc.sync.dma_start(bias_sb[:], biasr[:])

            # pre-zero the PSUM pool buffers: drains/TTs read pad columns no
            # matmul ever writes; virgin PSUM could be NaN.
            for _ in range(4):
                pz = ppool.tile([P, 2, BL, FDQ_CAP], f32, tag="ps")
                nc.scalar.mul(
                    pz[:].rearrange("p a b c -> p (a b c)"),
                    pz[:].rearrange("p a b c -> p (a b c)"),
                    0.0,
                )

            zflat = z_sb[:].rearrange("p t b -> p (t b)")
            trash = cpool.tile([P, FDQ_CAP], bf16)
            pending_act = []  # deferred scalar act-reduces: (u, Q, fq)
            pending_red = []  # deferred VE reduces for GpSimd quads: (u, Q)

            def flush_act():
                while pending_act:
                    pu, pq, pfq = pending_act.pop(0)
                    for tq in range(4):
                        t = 4 * pq + tq
                        for bb in range(BL):
                            nc.scalar.activation(
                                out=trash[:, 0:pfq],
                                in_=pu[:, tq, bb],
                                func=mybir.ActivationFunctionType.Copy,
                                accum_out=zflat[
                                    :, t * BL + bb : t * BL + bb + 1
                                ],
                            )

            for Q in range(NQUAD):
                fq = fdq[Q]
                f_bf = fpool.tile([P, 4, BL, fq], bf16, tag="f")
                u = upool.tile([P, 4, BL, fq], bf16, tag="u")
                eng = nc.gpsimd if Q in GPS_TT_QUADS else nc.vector
                s_q = s_sb[:, soff[Q] : soff[Q + 1]].rearrange(
                    "p (t d) -> p t d", t=4
                )
                for half in range(2):
                    # pair-granular PSUM units (2 banks, 4 in flight) keep
                    # the PE runway deep; two drains fill one quad buffer
                    ps = ppool.tile([P, 2, BL, FDQ_CAP], f32, tag="ps")
                    for tp in range(2):
                        tq = 2 * half + tp
                        t = 4 * Q + tq
                        bd, rmin, nr, xmin, xl = rects[t]
                        for ch in range(2):
                            nc.tensor.matmul(
                                ps[:, tp, :, 0 : nr * xl],
                                w_sb[:, t, ch, :],
                                x_bc[bd][ch][
                                    :, :,
                                    rmin : rmin + nr,
                                    xmin : xmin + xl,
                                ],
                                start=(ch == 0),
                                stop=(ch == 1),
                                skip_group_check=True,
                            )
                    hs = slice(2 * half, 2 * half + 2)
                    nc.scalar.copy(f_bf[:, hs], ps[:, :, :, 0:fq])
                    # half-granular TT starts right after its drain
                    eng.tensor_tensor(
                        out=u[:, hs],
                        in0=f_bf[:, hs],
                        in1=s_q[:, hs].unsqueeze(2).broadcast_to(
                            [P, 2, BL, fq]
                        ),
                        op=mybir.AluOpType.mult,
                    )
                    if half == 0:
                        flush_act()
                if Q in ACT_QUADS:
                    pending_act.append((u, Q, fq))
                elif Q in GPS_TT_QUADS:
                    # defer the VE reduce one quad so the in-order VE queue
                    # isn't head-blocked waiting on the slower GpSimd TT
                    pending_red.append((u, Q))
                else:
                    nc.vector.tensor_reduce(
                        out=z_sb[:, 4 * Q : 4 * Q + 4],
                        in_=u[:],
                        axis=mybir.AxisListType.X,
                        op=mybir.AluOpType.add,
                    )
                while len(pending_red) > 1:
                    pu, pq = pending_red.pop(0)
                    nc.vector.tensor_reduce(
                        out=z_sb[:, 4 * pq : 4 * pq + 4],
                        in_=pu[:],
                        axis=mybir.AxisListType.X,
                        op=mybir.AluOpType.add,
                    )

                if Q == NQUAD - 1 or Q == NQUAD // 2 - 1:
                    flush_act()
                    while pending_red:
                        pu, pq = pending_red.pop(0)
                        nc.vector.tensor_reduce(
                            out=z_sb[:, 4 * pq : 4 * pq + 4],
                            in_=pu[:],
                            axis=mybir.AxisListType.X,
                            op=mybir.AluOpType.add,
                        )
                    # epilogue on the completed half: z += bias; elu(z)+1
                    QT = NT // 2
                    h0 = 0 if Q == NQUAD // 2 - 1 else NT // 2
                    ht = slice(h0, h0 + QT)
                    hz = slice(h0 * BL, (h0 + QT) * BL)
                    zf = cpool.tile([P, QT * BL], f32, tag=f"zf{h0}")
                    ze = cpool.tile([P, QT * BL], f32, tag=f"ze{h0}")
                    nc.vector.tensor_tensor(
                        out=zf[:].rearrange("p (t b) -> p t b", b=BL),
                        in0=z_sb[:, ht],
                        in1=bias_sb[:, ht].unsqueeze(-1).broadcast_to(
                            [P, QT, BL]
                        ),
                        op=mybir.AluOpType.add,
                    )
                    # ze = exp(min(zf,0)) = Exp(-Relu(-zf)); zf = Relu(zf)
                    nc.scalar.activation(
                        ze[:], zf[:],
                        mybir.ActivationFunctionType.Relu, scale=-1.0,
                    )
                    nc.scalar.activation(
                        ze[:], ze[:],
                        mybir.ActivationFunctionType.Exp, scale=-1.0,
                    )
                    nc.scalar.activation(
                        zf[:], zf[:], mybir.ActivationFunctionType.Relu
                    )
                    nc.vector.tensor_add(zf[:], zf[:], ze[:])
                    nc.sync.dma_start(out[:, hz], zf[:])

    from concourse.library_overlay import lower_extended_insts

    lower_extended_insts(nc)
    _split_multi_waits(nc)
    nc.finalize()
    return nc


def _split_multi_waits(nc):
    """The walrus build in this environment only supports ONE sync-wait slot
    per instruction.  Hoist extra waits onto NoOps inserted just before the
    offending instruction (same engine, so sequencer order enforces them)."""
    import concourse.mybir as mybir
    import bass_rust

    for fn in nc.m.functions:
        for blk in fn.blocks:
            new_insts = []
            for ins in blk.instructions:
                si = getattr(ins, "sync_info", None)
                waits = list(si.on_wait) if si is not None else []
                if len(waits) > 1:
                    for j, w in enumerate(waits[:-1]):
                        nop = mybir.InstNoOp(name=f"{ins.name}-w{j}")
                        nop.engine = ins.engine
                        nop.sync_info = bass_rust.SyncInfo(
                            on_wait=[w], on_update=[]
                        )
                        new_insts.append(nop)
                    ins.sync_info = bass_rust.SyncInfo(
                        on_wait=[waits[-1]], on_update=list(si.on_update)
                    )
                new_insts.append(ins)
            blk.instructions[:] = new_insts


def _host_prep(x, mu, W, b):
    bf16 = ml_dtypes.bfloat16

    # --- per-neuron bilinear indices / weights ---
    gx = np.clip(mu[:, 0].astype(np.float64), -1.0, 1.0)
    gy = np.clip(mu[:, 1].astype(np.float64), -1.0, 1.0)
    ix = (gx + 1.0) * (Wd * 0.5) - 0.5
    iy = (gy + 1.0) * (H * 0.5) - 0.5
    x0 = np.floor(ix)
    y0 = np.floor(iy)
    wx1 = (ix - x0).astype(np.float32)
    wy1 = (iy - y0).astype(np.float32)
    wx0 = 1.0 - wx1
    wy0 = 1.0 - wy1
    x0i = np.clip(x0.astype(np.int32), 0, Wd - 2)
    y0i = np.clip(y0.astype(np.int32), 0, H - 2)

    # trimmed pixel bbox (cols/rows actually read, incl +1 halo)
    RMIN, RMAX = int(y0i.min()), int(y0i.max()) + 1
    CMIN, CMAX = int(x0i.min()), int(x0i.max()) + 1
    XC = CMAX - CMIN + 1

    # --- k-d clustering: first two splits on y => 4 y-bands of 16 tiles ---
    pts = np.stack([x0i, y0i], 1)

    def bbox_fd(idx):
        xs, ys = pts[idx, 0], pts[idx, 1]
        nr = ys.max() - ys.min() + 2
        xl = xs.max() - xs.min() + 2
        xl += xl & 1
        return nr * xl

    def kd(idx, axes, leaves):
        if len(idx) == P:
            leaves.append(idx)
            return
        h = len(idx) // 2
        if axes:
            ax, rest = axes[0], axes[1:]
        else:
            # pick the split axis minimizing the max child bbox area
            best = None
            for cand in (0, 1):
                o = np.argsort(pts[idx, cand], kind="stable")
                cost = max(bbox_fd(idx[o[:h]]), bbox_fd(idx[o[h:]]))
                if best is None or cost < best[0]:
                    best = (cost, cand)
            ax, rest = best[1], ()
        order = np.argsort(pts[idx, ax], kind="stable")
        kd(idx[order[:h]], rest, leaves)
        kd(idx[order[h:]], rest, leaves)

    leaves = []
    kd(np.arange(N), (1, 1, 1), leaves)  # first three splits on y
    # Re-split the first octant with one more forced y split so the first
    # two bands are 4 tiles each — a smaller first x DMA starts compute
    # earlier.
    oct0 = np.concatenate(leaves[:8])
    sub = []
    kd(oct0, (1,), sub)
    leaves = sub + leaves[8:]
    band_sizes = [4, 4] + [8] * 7
    # Within each band, sort leaves by bbox fd so quads group tiles of
    # similar size (cuts the quad max-pad on S'/DVE work).
    leaves2 = []
    lo = 0
    for bsz in band_sizes:
        grp = sorted(leaves[lo : lo + bsz], key=bbox_fd)
        leaves2.extend(grp)
        lo += bsz
    leaves = leaves2
    order = np.concatenate(leaves)
    y0s, x0s = y0i[order], x0i[order]
    w4 = np.stack(
        [wx0 * wy0, wx1 * wy0, wx0 * wy1, wx1 * wy1], axis=-1
    ).astype(np.float32)[order]

    # --- band row spans (in trimmed coords) incl halo ---
    NB = len(band_sizes)
    bandt0 = np.cumsum([0] + band_sizes).tolist()  # band tile offsets
    bandrows, bandoff, bandr0 = [], [], []
    off = 0
    for bd in range(NB):
        sl = slice(bandt0[bd] * P, bandt0[bd + 1] * P)
        rlo = int(y0s[sl].min()) - RMIN
        rhi = int(y0s[sl].max()) + 1 - RMIN
        bandr0.append(rlo)
        bandoff.append(off)
        bandrows.append(rhi - rlo + 1)
        off += rhi - rlo + 1

    # --- per-tile single rect (local to its band block) ---
    rects, fdt = [], []
    import bisect
    for t in range(NT):
        bd = bisect.bisect_right(bandt0, t) - 1
        sl = slice(t * P, (t + 1) * P)
        yy = y0s[sl] - RMIN - bandr0[bd]   # band-local row of y0
        xx = x0s[sl] - CMIN
        rmin = int(yy.min())
        nr = int(yy.max()) - rmin + 2   # corners reach y0+1
        xmin = int(xx.min())
        xl = int(xx.max()) - xmin + 2   # corners reach x0+1
        # matmul moving rows must be a whole number of 4-byte words
        gran = 4 if X_FP8 else 2
        xl = (xl + gran - 1) & ~(gran - 1)
        if xl > XC - xmin:
            xmin = XC - xl  # shift window left; interior guarantees room
        fd = nr * xl
        assert fd <= FDQ_CAP, (t, nr, xl, fd)
        rects.append((bd, rmin, nr, xmin, xl))
        fdt.append(fd)

    # quad padding for rectangular DVE ops
    fdq = [max(fdt[4 * q : 4 * q + 4]) for q in range(NT // 4)]
    fdq = [f + (f & 1) for f in fdq]
    soff = np.cumsum([0] + [4 * f for f in fdq]).tolist()
    ssz = soff[-1]

    # --- W quantization + per-neuron scale folded into S' ---
    Wp = W[order]  # [N, C] f32
    if W_FP8:
        e3m4 = ml_dtypes.float8_e3m4
        s = np.abs(Wp).max(axis=1) / 15.0  # per-neuron scale
        s = np.maximum(s, 1e-30)
        Wq = (Wp / s[:, None]).astype(e3m4)
        wdt = e3m4
    else:
        s = np.ones(N, dtype=np.float32)
        Wq = Wp.astype(bf16)
        wdt = bf16

    # --- S' (bilinear weights * s[n] over rect cols), pair-padded layout ---
    ss_np = np.zeros((P, ssz), dtype=np.float32)
    for t in range(NT):
        q, tp = t // 4, t % 4
        base = soff[q] + tp * fdq[q]
        bd, rmin, nr, xmin, xl = rects[t]
        sl = slice(t * P, (t + 1) * P)
        yy = y0s[sl] - RMIN - bandr0[bd] - rmin
        xx = x0s[sl] - CMIN - xmin
        sn = s[t * P : (t + 1) * P]
        for j in range(P):
            for (dr, dx, k) in ((0, 0, 0), (0, 1, 1), (1, 0, 2), (1, 1, 3)):
                r, xc = yy[j] + dr, xx[j] + dx
                assert 0 <= r < nr and 0 <= xc < xl, (t, j, r, xc)
                ss_np[j, base + r * xl + xc] += w4[t * P + j, k] * sn[j]
    ss_np = ss_np.astype(bf16)

    # --- W stationary: [c_part, t, ch, n] ---
    ws_np = np.ascontiguousarray(
        Wq.reshape(NT, P, 2, P)        # [t, n, ch, c_part]
        .transpose(3, 0, 2, 1)         # [c_part, t, ch, n]
        .reshape(P, NT * 2 * P)
    )
    biasr_np = np.ascontiguousarray(
        b[order].astype(np.float32).reshape(NT, P).T
    )

    # --- per-core x: band-major blocks [(band, ch), c_part, b, row, col] ---
    xdt = ml_dtypes.float8_e3m4 if X_FP8 else bf16
    xb = x.astype(xdt)[:, :, RMIN : RMAX + 1, CMIN : CMAX + 1]
    xts = []
    for cix in range(NCORES):
        xc_ = xb[cix * BL : (cix + 1) * BL]             # [BL, C, R, XC]
        xc5 = xc_.reshape(BL, 2, P, xc_.shape[2], XC)   # [b, ch, cp, r, x]
        blocks = []
        for bd in range(NB):
            for ch in range(2):
                blk = xc5[:, ch, :, bandr0[bd] : bandr0[bd] + bandrows[bd]]
                # [b, cp, rows, XC] -> [cp, b, rows, XC] flat per partition
                blocks.append(
                    np.ascontiguousarray(blk.transpose(1, 0, 2, 3)).reshape(
                        P, -1
                    )
                )
        xts.append(np.ascontiguousarray(np.concatenate(blocks, axis=1)))

    meta = {
        "rects": rects, "fdt": fdt, "fdq": fdq, "soff": soff, "ssz": ssz,
        "bandrows": bandrows, "bandoff": bandoff, "xc": XC,
        "bandt0": bandt0,
    }
    shared = {"ss": ss_np, "ws": ws_np, "biasr": biasr_np}
    in_maps = [{"xt": xts[cix], **shared} for cix in range(NCORES)]
    return in_maps, meta, order


def _run(prep, trace=False, **kwargs):
    global _PROGRAM
    from concourse import bass_utils

    in_maps, meta, order = prep
    if _PROGRAM is None:
        _PROGRAM = _build_program(meta)
    rr = bass_utils.run_bass_kernel_spmd(
        _PROGRAM, in_maps, core_ids=list(range(NCORES)), trace=trace, **kwargs
    )
    inv = np.empty(N, dtype=np.int64)
    inv[order] = np.arange(N)
    outs = []
    for cix in range(NCORES):
        o = np.asarray(rr.results[cix]["out"], dtype=np.float32)  # [P, NT*BL]
        o = o.reshape(P, NT, BL).transpose(2, 1, 0).reshape(BL, N)
        outs.append(o[:, inv])
    return np.concatenate(outs, axis=0), rr


def kernel(x, mu, W, b):
    prep = _host_prep(x, mu, W, b)
    out, _ = _run(prep)
    return out
